# revision 18
# baseline (speedup 1.0000x reference)
"""Trainium2 Bass kernel for nn_Attention_9242769622327.

Math: the reference computes
    qkv = x @ W1.T ; q,k,v = split(qkv)
    score = softmax(k^T v / 4, axis=-1)            # rows sum to 1
    attn  = softmax(einsum('bhnk,bhkc->bhnk', q/4, score), axis=-1)
          = softmax(q/4 * sum_c score)             # sum_c score == 1
          = softmax(q/4)                           # k/v are mathematically dead
    out   = attn @ W2.T
so only the q-projection (first E rows of W1), a per-head (64-wide) softmax,
and the output projection are needed.

Distribution: pure data-parallel over the 32768 = B*S rows; each of the 8
cores handles 4096 rows with the full weights. No collectives.

Precision: mm1 (q-projection) and the head-sum matmul run in fp8-e4m3
DoubleRow (2 fp8 MACs per PE cell per cycle -> half the matmul instructions
of bf16; measured same 213ns/MM issue rate at N=512).  The ~2.5% fp8 noise
on q is attenuated by the /4 + exp + softmax chain to ~1% on the output; the
head-sum noise is averaged down by the 64-wide sum.  mm2 must stay fp16:
quantizing attn to fp8 alone costs ~2.5% on the output (threshold is 2%).
W1q is pre-scaled by 32 so its entries (std 1/32) use the fp8 dynamic range;
the exp() activation applies scale 1/(4*32) to compensate.

On-chip layout is fully transposed (features on partitions, rows on the free
dim) so no on-chip transposes are needed anywhere:
    qT[n,m]  = sum_k W1qT[k,n] * xT[k,m]          (PE, fp8 DR, K=256/MM)
    u        = exp(qT/128)                        (ACT, PSUM->SBUF fp16)
    u8       = fp8(u)                             (DVE, pair-interleaved)
    s[g,m]   = sum_{n in head g} u8[n,m]          (PE fp8-DR w/ 0/1 selector)
    rcp      = 1/s                                (DVE reciprocal_approx_fast)
    rb[n,m]  = rcp[head(n),m]                     (PE matmul w/ selector^T)
    aT       = u * rb                             (DVE)
    outT[j,m]= sum_n W2T[n,j] * aT[n,m]           (PE, fp16)

Stripes are software-pipelined: stripe ms emits [32 mm1-DR][8 rb(ms-1)]
[32 mm2(ms-1) j=0-3][4 sel-DR][32 mm2(ms-1) j=4-7]; the Tile scheduler
interleaves these so the PE streams at its ~215ns/MM issue floor (N=512).
Engine-FIFO ordering is load-bearing: on the DVE, the at-muls (which mm2
blocks on) are emitted before the u8 conversions (whose consumer, the sel
head-sum, runs ~8us later); o_ps drains alternate ACT (j=0-3) / DVE (j=4-7)
so neither queue falls behind the PSUM bank rotation.  PSUM: 3 q banks
(so mm1 never waits on the ACT exp drain), 1 s, 2 rb, 2 out.  w2/selt DMAs
are deferred behind the stripe-0/1 x + w1 loads so the first mm1 block
starts as early as possible; 8 throwaway matmuls on memset scratch warm
the PE HAM clock gate (1.2 -> 2.4 GHz) during that DMA window.  x/w1 are
host-packed so every DMA tile is one contiguous DRAM block.

Measured on 8 axon trn2 cores: ~208.2us, rel err 9.85e-3 (vs 282.4us for
the all-fp16 baseline; PE-stream floor for this structure is ~186us + ~12us
fixed head (preamble + first DMAs) + ~5us tail).  Note: runs occasionally
land ~20% slower when the chip is in the P0 power state (PE at 2.0 instead
of 2.4 GHz) — that is environment noise, not kernel-dependent.
"""

import sys

sys.path.insert(0, "/opt/trn_rl_repo")

import numpy as np
import ml_dtypes

import concourse.bass as bass
import concourse.bacc as bacc
import concourse.tile as tile
from concourse import mybir
from concourse.bass_utils import run_bass_kernel_spmd

BF16 = mybir.dt.float16  # fp16: same PE rate as bf16, 10-bit mantissa
FP8 = mybir.dt.float8e4
F32 = mybir.dt.float32
AF = mybir.ActivationFunctionType
DR = mybir.MatmulPerfMode.DoubleRow

N_CORES = 8
B, S, E = 4, 8192, 1024
HEADS, HEAD_DIM = 16, 64
M_TOTAL = B * S                # 32768
M_CORE = M_TOTAL // N_CORES    # 4096 rows per core
MS = 512                       # m-stripe width (moving free dim / PSUM bank)
N_STRIPES = M_CORE // MS       # 8
KP = E // 256                  # 4 DoubleRow contraction pair-chunks
NC_ = E // 128                 # 8 feature chunks
W1_SCALE = 32.0                # pre-scale on W1q before fp8 quantization

_BF = np.float16
_F8 = ml_dtypes.float8_e4m3fn


def build_nc() -> bass.Bass:
    nc = bacc.Bacc("TRN2", debug=False)

    # x/w1 are pre-packed on host so every DMA tile is one contiguous
    # block (1-2KB per-partition lines instead of 512B strided runs)
    xt8 = nc.dram_tensor("xt8", [KP, N_STRIPES, 128, 2 * MS], FP8, kind="ExternalInput")
    w18 = nc.dram_tensor("w18", [KP, 128, 2 * E], FP8, kind="ExternalInput")
    w2t = nc.dram_tensor("w2t", [E, E], BF16, kind="ExternalInput")
    sel8 = nc.dram_tensor("sel8", [128, KP * 2 * HEADS], FP8, kind="ExternalInput")
    selt = nc.dram_tensor("selt", [128, NC_ * 128], BF16, kind="ExternalInput")
    outT = nc.dram_tensor("outT", [E, M_CORE], BF16, kind="ExternalOutput")

    w2_v = w2t[:, :].rearrange("(c p) j -> p c j", p=128)   # [128, 8, 1024]

    with tile.TileContext(nc) as tc:
        with (
            tc.tile_pool(name="weights", bufs=1) as wpool,
            tc.tile_pool(name="xt", bufs=N_STRIPES) as xpool,
            tc.tile_pool(name="u", bufs=16) as upool,
            tc.tile_pool(name="u8", bufs=2) as u8pool,
            tc.tile_pool(name="at", bufs=16) as apool,
            tc.tile_pool(name="small", bufs=3) as spool,
            tc.tile_pool(name="ostage", bufs=8) as opool,
            tc.tile_pool(name="ps_q", bufs=3, space="PSUM") as psq,
            tc.tile_pool(name="ps_s", bufs=1, space="PSUM") as pss,
            tc.tile_pool(name="ps_rb", bufs=2, space="PSUM") as psrb,
            tc.tile_pool(name="ps_o", bufs=2, space="PSUM") as pso,
        ):
            # Warm the PE's HAM clock gate with throwaway matmuls on memset
            # scratch while the first weight/x DMAs are in flight, so the
            # first real matmuls run at 2.4 GHz instead of 1.2.  ~8 cold MMs
            # x 427ns covers the ~3.4us HAM window.
            warm_sb = wpool.tile([128, MS], BF16, name="warm_sb")
            nc.gpsimd.memset(warm_sb[:], 0.0)
            warm_ps = psq.tile([128, MS], F32, tag="q", name="warm_ps")
            for _ in range(8):
                nc.tensor.matmul(
                    warm_ps[:], warm_sb[:, 0:128], warm_sb[:], start=True, stop=True
                )

            # Stripe-0-critical loads first: w1 pair-chunks interleaved with
            # stripe-0 x pair-chunks, then the tiny sel8.  w2/selt are
            # deferred until after stripe 1's x loads (they aren't read until
            # stripe 0's normalization/output projection, ~25us in).
            w1_t = []
            xt0 = []
            for t in range(KP):
                w = wpool.tile([128, 2, E], FP8, tag=f"w1_{t}", name=f"w1t{t}")
                nc.sync.dma_start(w[:], w18[t, :, :])
                w1_t.append(w)
                tx = xpool.tile([128, 2, MS], FP8, tag=f"xt_{t}", name=f"xt0_{t}")
                nc.sync.dma_start(tx[:], xt8[t, 0, :, :])
                xt0.append(tx)
            sel8_t = wpool.tile([128, KP, 2, HEADS], FP8, name="sel8_t")
            nc.sync.dma_start(
                sel8_t[:],
                sel8[:, :].rearrange("p (t two h) -> p t two h", two=2, h=HEADS),
            )

            w2_k = [
                wpool.tile([128, E], BF16, tag=f"w2_{ci}", name=f"w2k{ci}")
                for ci in range(NC_)
            ]
            selt_t = wpool.tile([128, NC_, 128], BF16, name="selt_t")

            # Software pipeline over stripes: while stripe ms runs its
            # q-projection (mm1) + exp + head-sum on the PE, stripe ms-1's
            # normalization (rb broadcast matmul + DVE mul) and output
            # projection (mm2) are interleaved so the PE never waits on the
            # softmax chain.
            prev_u = None       # u tiles of stripe ms-1
            prev_u8 = None      # fp8 pair-tiles of stripe ms-1
            prev_rcp = None     # reciprocal head-sums of stripe ms-1 (fp16)
            prev_ms = -1

            def emit_selrcp(pu8):
                """4-MM fp8-DR head-sum block + reciprocal + 128-row pad.
                Emitted at the HEAD of the next stripe's DR region so the
                fp16->DR boundary lands on this hazard-free group (mm1's
                accumulation-group starts otherwise eat a ~1-slot LDW hole)
                and rcp is ready long before the rb matmuls need it."""
                s_ps = pss.tile([HEADS, MS], F32, tag="s", name="s_ps")
                for t in range(KP):
                    nc.tensor.matmul(
                        s_ps[:],
                        sel8_t[:, t, :, :],
                        pu8[t][:],
                        start=(t == 0),
                        stop=(t == KP - 1),
                        perf_mode=DR,
                    )
                rcp32 = spool.tile([HEADS, MS], F32, tag="rcp32", name="rcp32")
                nc.vector.reciprocal_approx_fast(rcp32[:], s_ps[:])
                rcp_t = spool.tile([128, MS], BF16, tag="rcp", name="rcp_t")
                nc.gpsimd.memset(rcp_t[:], 0.0)
                nc.scalar.copy(rcp_t[0:HEADS, :], rcp32[:])
                return rcp_t

            def emit_norm(pu, prcp):
                """rb broadcast matmuls (PE, K padded to 128 so LDWEIGHTS
                overlaps like the main GEMM blocks) + DVE muls."""
                ats = []
                for ci in range(NC_):
                    rb_ps = psrb.tile([128, MS], F32, tag="rb", name="rb_ps")
                    nc.tensor.matmul(
                        rb_ps[:], selt_t[:, ci, :], prcp[:], start=True, stop=True
                    )
                    at_t = apool.tile([128, MS], BF16, tag="at", name="at_t")
                    nc.vector.tensor_mul(at_t[:], pu[ci][:], rb_ps[:])
                    ats.append(at_t)
                return ats

            def emit_tail(at_list, ms, js, copy_engine):
                """Emit mm2 + store for a finished stripe (at tiles ready).
                copy_engine picks which engine drains o_ps so the ACT and DVE
                FIFOs each stay ahead of the PSUM-bank rotation."""
                for j in js:
                    o_ps = pso.tile([128, MS], F32, tag="o", name="o_ps")
                    for ci in range(NC_):
                        nc.tensor.matmul(
                            o_ps[:],
                            w2_k[ci][:, j * 128:(j + 1) * 128],
                            at_list[ci][:],
                            start=(ci == 0),
                            stop=(ci == NC_ - 1),
                        )
                    o_t = opool.tile([128, MS], BF16, tag="ost", name="o_t")
                    if copy_engine == "act":
                        nc.scalar.copy(o_t[:], o_ps[:])
                    else:
                        nc.vector.tensor_scalar_mul(o_t[:], o_ps[:], 1.0)
                    nc.sync.dma_start(
                        outT[j * 128:(j + 1) * 128, ms * MS:(ms + 1) * MS], o_t[:]
                    )

            for ms in range(N_STRIPES):
                if ms == 0:
                    xt_k = xt0
                else:
                    xt_k = []
                    for t in range(KP):
                        tx = xpool.tile(
                            [128, 2, MS], FP8, tag=f"xt_{t}", name=f"xt{ms}_{t}"
                        )
                        nc.sync.dma_start(tx[:], xt8[t, ms, :, :])
                        xt_k.append(tx)
                if ms == 1:
                    # deferred bulk loads (needed from stripe-0 norm onwards)
                    nc.sync.dma_start(
                        selt_t[:],
                        selt[:, :].rearrange("p (c q) -> p c q", q=128),
                    )
                    for ci in range(NC_):
                        nc.sync.dma_start(w2_k[ci][:], w2_v[:, ci, :])

                # ---- DR region: head-sum of ms-1 first, then mm1(ms) ----
                prev_rcp = emit_selrcp(prev_u8) if prev_u8 is not None else None

                u_tiles = []
                for ci in range(NC_):
                    q_ps = psq.tile([128, MS], F32, tag="q", name="q_ps")
                    for t in range(KP):
                        nc.tensor.matmul(
                            q_ps[:],
                            w1_t[t][:, :, ci * 128:(ci + 1) * 128],
                            xt_k[t][:],
                            start=(t == 0),
                            stop=(t == KP - 1),
                            perf_mode=DR,
                        )
                    u_t = upool.tile([128, MS], BF16, tag="u", name="u_t")
                    nc.scalar.activation(
                        u_t[:], q_ps[:], AF.Exp, scale=1.0 / (4.0 * W1_SCALE)
                    )
                    u_tiles.append(u_t)

                # ---- stripe ms-1 normalization (hides exp latency).
                # Emitted BEFORE the u8 conversions: the DVE queue is FIFO,
                # and mm2 blocks on the at tiles, while the sel head-sum (the
                # u8 consumer) runs a full stripe later. ----
                at_tiles = emit_norm(prev_u, prev_rcp) if prev_rcp is not None else None

                # pair-interleaved fp8 copies for stripe ms's DR head-sum
                # (consumed at the head of iteration ms+1)
                u8_tiles = []
                for ci in range(NC_):
                    if ci % 2 == 0:
                        u8_t = u8pool.tile(
                            [128, 2, MS], FP8, tag=f"u8_{ci // 2}", name="u8_t"
                        )
                        u8_tiles.append(u8_t)
                    nc.vector.tensor_scalar_mul(
                        u8_tiles[ci // 2][:, ci % 2, :], u_tiles[ci][:], 1.0
                    )

                # ---- stripe ms-1 output projection ----
                if at_tiles is not None:
                    emit_tail(at_tiles, prev_ms, range(0, NC_ // 2), "act")
                    emit_tail(at_tiles, prev_ms, range(NC_ // 2, NC_), "dve")
                prev_u, prev_u8, prev_ms = u_tiles, u8_tiles, ms

            # epilogue: last stripe's head-sum + normalization + output proj
            rcp_t = emit_selrcp(prev_u8)
            at_tiles = emit_norm(prev_u, rcp_t)
            emit_tail(at_tiles, prev_ms, range(0, NC_ // 2), "act")
            emit_tail(at_tiles, prev_ms, range(NC_ // 2, NC_), "dve")
    nc.compile()
    return nc


_NC_CACHE = None
LAST_RESULT = None


def _ensure_ntff_hook():
    """bass_utils' axon trace path needs antenv.axon_hooks, which this
    container's antenv lacks. Provide it + register the ctypes NTFF hook."""
    import types

    try:
        from antenv.axon_hooks import get_axon_ntff_profile_hook  # noqa: F401
        return True
    except ImportError:
        pass
    try:
        import antenv
        from trn_agent_boot.trn_boot import _ntff_profile_via_ctypes

        m = types.ModuleType("antenv.axon_hooks")
        state = {"hook": None}
        m.set_axon_ntff_profile_hook = lambda h: state.__setitem__("hook", h)
        m.get_axon_ntff_profile_hook = lambda: state["hook"]
        sys.modules["antenv.axon_hooks"] = m
        antenv.axon_hooks = m
        m.set_axon_ntff_profile_hook(
            _ntff_profile_via_ctypes("/opt/axon/libaxon_pjrt.so")
        )
        return True
    except Exception as e:  # pragma: no cover
        print(f"ntff hook injection failed: {e}")
        return False


def _selectors():
    # head index of global feature n is n // 64; pair-chunk t group i covers
    # chunk ci = 2t+i, i.e. heads 2ci (partitions 0..63) and 2ci+1 (64..127).
    sel8 = np.zeros((128, KP, 2, HEADS), np.float32)
    for t in range(KP):
        for i in range(2):
            ci = 2 * t + i
            sel8[:64, t, i, 2 * ci] = 1.0
            sel8[64:, t, i, 2 * ci + 1] = 1.0
    selt = np.zeros((128, NC_, 128), np.float32)  # K padded to 128, rows 16+ zero
    for ci in range(NC_):
        selt[2 * ci, ci, :64] = 1.0
        selt[2 * ci + 1, ci, 64:] = 1.0
    return (
        np.ascontiguousarray(sel8.reshape(128, KP * 2 * HEADS)).astype(_F8),
        np.ascontiguousarray(selt.reshape(128, NC_ * 128)).astype(_BF),
    )


def kernel(x, W1, W2, heads, trace=False):
    global _NC_CACHE, LAST_RESULT
    x = np.asarray(x, dtype=np.float32)
    W1 = np.asarray(W1, dtype=np.float32)
    W2 = np.asarray(W2, dtype=np.float32)

    X = x.reshape(M_TOTAL, E)
    X8T = np.ascontiguousarray(X.T).astype(_F8)           # [E, M_TOTAL]
    w18 = np.ascontiguousarray((W1[:E, :] * W1_SCALE).T).astype(_F8)  # [k, n]
    w2t = np.ascontiguousarray(W2.T).astype(_BF)          # [n, j] = W2[j, n]
    sel8, selt = _selectors()

    w18p = np.ascontiguousarray(
        w18.reshape(KP, 2, 128, E).transpose(0, 2, 1, 3).reshape(KP, 128, 2 * E)
    )
    in_maps = []
    for c in range(N_CORES):
        xt_c = X8T[:, c * M_CORE:(c + 1) * M_CORE]
        xt_p = np.ascontiguousarray(
            xt_c.reshape(KP, 2, 128, N_STRIPES, MS)
            .transpose(0, 3, 2, 1, 4)
            .reshape(KP, N_STRIPES, 128, 2 * MS)
        )
        in_maps.append(
            {"xt8": xt_p, "w18": w18p, "w2t": w2t, "sel8": sel8, "selt": selt}
        )

    if _NC_CACHE is None:
        _NC_CACHE = build_nc()

    if trace:
        trace = _ensure_ntff_hook()

    res = run_bass_kernel_spmd(_NC_CACHE, in_maps, list(range(N_CORES)), trace=trace)
    LAST_RESULT = res

    OT = np.concatenate(
        [np.asarray(res.results[c]["outT"]).astype(np.float32) for c in range(N_CORES)],
        axis=1,
    )
    return np.ascontiguousarray(OT.T).reshape(B, S, E)
